# revision 33
# baseline (speedup 1.0000x reference)
"""DynamicW8A8Int8Linear on 8 Trainium2 NeuronCores (Bass/Tile).

Column-parallel: each core gets the full activation x [8192, 4096] and a
1536-wide shard of weight / weight_scale / bias; it computes its
[8192, 1536] slice of the output. No communication.

The int8 GEMM acc = x_q @ w.T runs in fp16 on the Tensor engine (exact:
x_q, w are integers, products exact in fp32 PSUM).

Per 128-token m-tile:
  - DMA x tile [128, 4096] fp32 (SP HWDGE ring)
  - DVE: amax = max|x| over K; xs = max(amax,1e-8)/127; inv = 1/xs
  - Act: tq = x*inv + 1536 -> fp16 (the fp16 cast rounds to integer+1536
    exactly: spacing 1 in [1024, 2048), round-to-nearest-even like the
    reference's jnp.round)
  - PE: native transposes of tq into K-major xqT (fp16 PSUM, groups of 4
    per bank), Act copies PSUM -> SBUF.  (A DMA-xbar transpose variant,
    TDMA=1, measured slower: 256B packets swamp the DMA engines.)
  - PE: 96 accumulating fp16 matmuls (32 k-tiles x 3 psum banks); the
    +1536 offset rides through the GEMM
  - DVE/GpSimd epilogue (software-pipelined one tile behind, per psum
    bank): out = acc*xs*ws + wc*xs + b, where wc = -1536*colsum(w)*ws
    (host-precomputed) removes the activation offset exactly
Ring assignment: x-in + weights on the SP HWDGE ring, out on the Act
HWDGE ring -- so x(i+1) never queues behind out(i-1), which was the
per-tile 577ns PE stall in the previous revision.
The int8 weight shard is host-transposed to K-major and host-cast to
fp16 (12.6MB/core); it loads via the fast hardware DGE in 8 chunks
behind x(0) and stays resident in SBUF across all 64 m-tiles.  ~56
dummy matmuls at t=0 ramp the PE HAM clock gate to 2.4 GHz while the
first x tile + weights are in flight, so real matmuls start warm.
"""
import os

import sys
from contextlib import ExitStack

import numpy as np

for p in ("/opt/trn_rl_repo", "/opt/pypackages"):
    if p not in sys.path:
        sys.path.append(p)

import ml_dtypes
import orjson
import bass_rust
import concourse.bass as bass
import concourse.mybir as mybir
import concourse.tile as tile
from concourse.masks import make_identity
from concourse.vector_clock import ScopedClock
from concourse.bass_utils import run_bass_kernel_spmd

# ---------------------------------------------------------------------------
# Workaround for the walrus build here, which accepts at most ONE sem-wait per
# instruction ("Too many sync wait commands" in setupSyncWait): split the Tile
# end-drain at emission time, and hoist excess waits from any instruction onto
# injected same-engine NoOps at serialization time (program order on the same
# engine makes that semantically identical).
# ---------------------------------------------------------------------------
MAX_WAITS = 1


def _drain_and_barrier_split(self, tick_clock, wait_clock):
    nc = self.nc
    drain_inst = nc.sync.drain()
    wait_clock.add_sem_waits(drain_inst.ins, ScopedClock({None: tick_clock.global_clock}))
    si = drain_inst.ins.sync_info
    waits = list(si.on_wait) if si is not None and si.on_wait else []
    if len(waits) > MAX_WAITS:
        si.on_wait = waits[:MAX_WAITS]
        drain_inst.ins.sync_info = si
        rest = waits[MAX_WAITS:]
        while rest:
            extra = nc.sync.drain()
            extra.ins.sync_info = bass_rust.SyncInfo(
                on_wait=rest[:MAX_WAITS], on_update=[])
            rest = rest[MAX_WAITS:]
    nc.all_engine_barrier()
    assert self.sems is not None
    popped = nc._tile_sem_poison_stack.pop()
    assert popped is self._sem_poison
    nc.clear_and_free_semaphores(list(self.sems.allocated().values()))
    nc.all_engine_barrier()


_split_counter = [0]


def _split_waits_json(raw: bytes) -> bytes:
    j = orjson.loads(raw)
    changed = [False]

    def fix_block(b):
        ins_list = b.get("instructions")
        if ins_list:
            new_list = []
            for ins in ins_list:
                si = ins.get("sync_info")
                waits = (si or {}).get("on_wait") or []
                if len(waits) > MAX_WAITS:
                    changed[0] = True
                    for w in waits[:-MAX_WAITS]:
                        _split_counter[0] += 1
                        new_list.append({
                            "name": f"WSPLIT-{_split_counter[0]}",
                            "opcode": "NoOp",
                            "engine": ins["engine"],
                            "ins": [],
                            "outs": [],
                            "sync_info": {"on_update": [], "on_wait": [w]},
                        })
                    si["on_wait"] = waits[-MAX_WAITS:]
                new_list.append(ins)
            b["instructions"] = new_list
        for sub in (b.get("blocks") or []):
            fix_block(sub)

    for fn in j.get("functions", []):
        for b in (fn.get("blocks") or []):
            fix_block(b)
    if not changed[0]:
        return raw
    return orjson.dumps(j)


_orig_to_json_bytes = bass.Bass.to_json_bytes


def _to_json_bytes_split(self) -> bytes:
    return _split_waits_json(_orig_to_json_bytes(self))


tile.TileContext._drain_and_barrier = _drain_and_barrier_split
bass.Bass.to_json_bytes = _to_json_bytes_split

# ---------------------------------------------------------------------------
# Kernel
# ---------------------------------------------------------------------------
P = 128
MAGIC16 = 1536.0    # 1.5 * 2**10: fp16 add rounds to nearest-even integer
FREE = 512          # psum bank width (fp32)

M_FULL, K_DIM, N_FULL = 8192, 4096, 12288
N_CORES = 8
NS = N_FULL // N_CORES  # 1536 out_features per core
KT = K_DIM // P         # 32 k-tiles
NB = NS // FREE         # 3 psum banks
N_WARMUP = 25           # dummy matmuls (after the 9 broadcast matmuls) that
                        # keep the PE busy until the tile-0 quant chain lands

f32 = mybir.dt.float32
fp16 = mybir.dt.float16

ALU = mybir.AluOpType
ACT = mybir.ActivationFunctionType


def _emit(ctx: ExitStack, tc: tile.TileContext, x_ap, wt_ap, ws_ap,
          b_ap, wc_ap, out_ap):
    nc = tc.nc
    M, K = x_ap.shape
    MT = M // P

    const = ctx.enter_context(tc.tile_pool(name="const", bufs=1))

    # Per-channel scale / offset / bias: DMA only the [1, 1536] rows (18KB)
    # and broadcast to all 128 partitions ON-CHIP via K=1 fp32 matmuls with a
    # ones-vector (PE is idle at startup anyway and this doubles as HAM
    # warm-up).  A partition_broadcast DMA would pull 2.3MB through HBM right
    # when the weight load is bandwidth-critical.
    # The rows land in partition 0 of the destination tiles themselves (no
    # extra SBUF); the broadcast matmul reads row 0 before the full-tile
    # copy overwrites it.
    wsb = const.tile([P, NS], f32)
    bb = const.tile([P, NS], f32)
    wcb = const.tile([P, NS], f32)
    nc.sync.dma_start(wsb[0:1, :], ws_ap.rearrange("n o -> o n"))
    nc.sync.dma_start(bb[0:1, :], b_ap[None, :])
    nc.sync.dma_start(wcb[0:1, :], wc_ap[None, :])

    xpool = ctx.enter_context(tc.tile_pool(name="x", bufs=2))
    qpool = ctx.enter_context(tc.tile_pool(name="q", bufs=2))
    qtpool = ctx.enter_context(tc.tile_pool(name="qt", bufs=2))
    opool = ctx.enter_context(tc.tile_pool(name="o", bufs=2))
    # bufs=3: xs(mi) is read by _epilogue(mi), which is emitted one iteration
    # AFTER _quant(mi+2) allocates -- with 2 bufs the overwrite would be
    # emitted before the read and corrupt the scales.
    spool = ctx.enter_context(tc.tile_pool(name="s", bufs=3))
    mpsum = ctx.enter_context(tc.tile_pool(name="mpsum", bufs=2, space="PSUM"))
    tpsum = ctx.enter_context(tc.tile_pool(name="tpsum", bufs=2, space="PSUM"))
    ident = const.tile([P, P], fp16)
    make_identity(nc, ident)

    # Dummy data for PE warm-up matmuls.
    dummy = const.tile([P, FREE], fp16)
    nc.vector.memset(dummy[:], 1.0)

    # Prewarm the Act engine's activation table (the first ACTIVATE pays a
    # one-time ~1.3us ACT_TABLE_LOAD; do it at t~0, not on tq(0)'s critical
    # path).  Writes a scratch column so the warm-up matmuls reading `dummy`
    # aren't serialized behind it.
    prew = const.tile([P, 1], fp16)
    nc.scalar.activation(prew[:], dummy[:, 0:1], ACT.Copy)

    # SP-ring order: x0, weight chunk 0, x1, chunks 1-7.  x0 + chunk 0 gate
    # the first matmuls; the rest stream behind while tile 0 computes.
    wT = const.tile([P, KT, NS], fp16)
    wt_r = wt_ap.rearrange("(kt p) n -> p kt n", p=P)

    def _wchunk(ck):
        nc.sync.dma_start(wT[:, ck * 4:(ck + 1) * 4, :],
                          wt_r[:, ck * 4:(ck + 1) * 4, :])

    xt_pre = []
    xt0 = xpool.tile([P, K], f32, tag="xt", name="xt0")
    nc.sync.dma_start(xt0[:], x_ap[0:P, :])
    xt_pre.append(xt0)
    _wchunk(0)
    xt1 = xpool.tile([P, K], f32, tag="xt", name="xt1")
    nc.sync.dma_start(xt1[:], x_ap[P:2 * P, :])
    xt_pre.append(xt1)
    for ck in range(1, 8):
        _wchunk(ck)

    # PE warm-up: the HAM clock gate needs ~3.4us of sustained PE activity to
    # unthrottle 1.2 -> 2.4 GHz.  Run dummy matmuls while the first x tile /
    # weights are still in flight so real matmuls start warm.  The output
    # borrows an mpsum rotation slot (WAW with the first real m-tile's bank,
    # which starts later anyway) so no extra PSUM bank is needed.
    wps = mpsum.tile([P, FREE], f32, tag="mps0", name="wps")
    for wi in range(N_WARMUP):
        nc.tensor.matmul(wps[:], dummy[:, 0:P], dummy[:], start=True, stop=True)

    # On-chip broadcast of the epilogue vectors (after the dummies, so the fp32
    # matmuls run at warm clock): 9 K=1 fp32 matmuls + Act PSUM->SBUF copies.
    ones1 = const.tile([1, P], f32)
    nc.vector.memset(ones1[:], 1.0)
    for ti, dst in enumerate((wsb, bb, wcb)):
        for nb_i in range(NB):
            sl = slice(nb_i * FREE, (nb_i + 1) * FREE)
            bps = mpsum.tile([P, FREE], f32, tag=f"mps{nb_i}",
                             name=f"bc{ti}_{nb_i}")
            nc.tensor.matmul(bps[:], ones1[:], dst[0:1, sl], start=True,
                             stop=True)
            nc.scalar.activation(dst[:, sl], bps[:], ACT.Copy)

    def _epilogue_bank(pmi, pbank, pxs, nb_i):
        # out = acc * xs * ws + wc * xs + b  (wc removes the +1536 offset:
        # wc = -1536 * colsum(w) * ws, host-precomputed).  Out-DMA on the
        # gpsimd SWDGE ring: keeps the Act queue (tq + transpose copies) and
        # the SP ring (x in) free of epilogue work.
        sl = slice(nb_i * FREE, (nb_i + 1) * FREE)
        ot = opool.tile([P, FREE], f32, tag=f"ot{nb_i}", name=f"ot{pmi}_{nb_i}")
        nc.vector.scalar_tensor_tensor(
            ot[:], pbank[:], pxs[:, 0:1], wsb[:, sl],
            op0=ALU.mult, op1=ALU.mult,
        )
        nc.vector.scalar_tensor_tensor(
            ot[:], wcb[:, sl], pxs[:, 0:1], ot[:], op0=ALU.mult, op1=ALU.add,
        )
        nc.gpsimd.tensor_tensor(ot[:], ot[:], bb[:, sl], op=ALU.add)
        nc.gpsimd.dma_start(out_ap[pmi * P:(pmi + 1) * P, sl], ot[:])

    def _epilogue(pmi, pbanks, pxs):
        for nb_i in range(NB):
            _epilogue_bank(pmi, pbanks[nb_i], pxs, nb_i)

    def _quant(mi):
        """x DMA + per-token quant chain for tile mi; returns (xs, tq, xqT)."""
        if mi < 2:
            xt = xt_pre[mi]
        else:
            xt = xpool.tile([P, K], f32, tag="xt", name=f"xt{mi}")
            # mi 2/3 go via the Act HWDGE ring: on the SP ring they would
            # queue behind all 12.6MB of weight chunks and land too late for
            # the tile-1/2 transpose weave.
            eng = nc.scalar if mi in (2, 3) else nc.sync
            eng.dma_start(xt[:], x_ap[mi * P:(mi + 1) * P, :])

        # per-token quant params (DVE)
        amax = spool.tile([P, 1], f32, tag="amax", name=f"amax{mi}")
        nc.vector.tensor_reduce(
            amax[:], xt[:], axis=mybir.AxisListType.X,
            op=ALU.max, apply_absolute_value=True,
        )
        xs = spool.tile([P, 1], f32, tag="xs", name=f"xs{mi}")
        nc.vector.tensor_scalar(
            xs[:], amax[:], 1e-8, 1.0 / 127.0, op0=ALU.max, op1=ALU.mult,
        )
        inv = spool.tile([P, 1], f32, tag="inv", name=f"inv{mi}")
        nc.vector.reciprocal(inv[:], xs[:])

        # tq = x*inv + 1536 -> fp16: the fp16 cast rounds to integer+1536
        # exactly (spacing 1 in [1024, 2048)).  Two halves so the first
        # transpose groups can start before the whole row is quantized.
        tq = qpool.tile([P, K], fp16, tag="tq", name=f"tq{mi}")
        for th in range(2):
            hs = slice(th * (K // 2), (th + 1) * (K // 2))
            nc.scalar.activation(tq[:, hs], xt[:, hs], ACT.Copy, bias=MAGIC16,
                                 scale=inv[:, 0:1])
        xqT = qtpool.tile([P, KT, P], fp16, tag="xqT", name=f"xqT{mi}")
        return (xs, tq, xqT)

    def _tgroup(mi, tq, xqT, g):
        """PE-transpose k-tiles 8g..8g+7 of tq into PSUM; Act copies to xqT."""
        pt = tpsum.tile([P, 8, P], fp16, tag="tps", name=f"tps{mi}_{g}")
        for jj in range(8):
            c = g * 8 + jj
            nc.tensor.transpose(
                pt[:, jj, :], tq[:, c * P:(c + 1) * P], ident[:],
            )
        nc.scalar.activation(xqT[:, g * 8:(g + 1) * 8, :], pt[:], ACT.Copy)

    # Tile 0's quant + transposes stand alone (nothing to weave them into).
    q = {0: _quant(0)}
    for g in range(4):
        _tgroup(0, q[0][1], q[0][2], g)

    # Transpose groups of tile mi+1 are woven into the tail of tile mi's
    # matmul stream (after kt 26/28/30/31): the PE pays the 32x128-col
    # transpose cost but its Act PSUM->SBUF copies fully hide under matmuls,
    # so there is no copy-latency stall and no idle at the tile boundary.
    WEAVE_AT = {26: 0, 28: 1, 30: 2}
    prev = None
    for mi in range(MT):
        xs, tq, xqT = q.pop(mi)
        if mi + 1 < MT:
            q[mi + 1] = _quant(mi + 1)
            nxt = q[mi + 1]

        if prev is not None:
            _epilogue(*prev)

        # main GEMM (fp16 exact): acc[m, n] += xqT[p, kt, m] * wT[p, kt, n]
        banks = [
            mpsum.tile([P, FREE], f32, tag=f"mps{nb_i}", name=f"mps{mi}_{nb_i}")
            for nb_i in range(NB)
        ]
        if mi < MT - 1:
            for kt in range(KT):
                lhsT = xqT[:, kt, :]
                for nb_i in range(NB):
                    nc.tensor.matmul(
                        banks[nb_i][:], lhsT,
                        wT[:, kt, nb_i * FREE:(nb_i + 1) * FREE],
                        start=(kt == 0), stop=(kt == KT - 1),
                    )
                if kt in WEAVE_AT:
                    _tgroup(mi + 1, nxt[1], nxt[2], WEAVE_AT[kt])
            _tgroup(mi + 1, nxt[1], nxt[2], 3)
            prev = (mi, banks, xs)
        else:
            # Last tile: bank-major so each bank's epilogue + out-DMA overlaps
            # the next bank's matmuls -- shortens the kernel tail.
            for nb_i in range(NB):
                for kt in range(KT):
                    nc.tensor.matmul(
                        banks[nb_i][:], xqT[:, kt, :],
                        wT[:, kt, nb_i * FREE:(nb_i + 1) * FREE],
                        start=(kt == 0), stop=(kt == KT - 1),
                    )
                _epilogue_bank(mi, banks[nb_i], xs, nb_i)
            prev = None
    if prev is not None:
        _epilogue(*prev)


def _build_nc(m_rows=M_FULL):
    nc = bass.Bass()
    x = nc.dram_tensor("x", (m_rows, K_DIM), f32, kind="ExternalInput")
    wt = nc.dram_tensor("wt", (K_DIM, NS), fp16, kind="ExternalInput")
    ws = nc.dram_tensor("ws", (NS, 1), f32, kind="ExternalInput")
    b = nc.dram_tensor("b", (NS,), f32, kind="ExternalInput")
    wc = nc.dram_tensor("wc", (NS,), f32, kind="ExternalInput")
    out = nc.dram_tensor("out", (m_rows, NS), f32, kind="ExternalOutput")
    with tile.TileContext(nc) as tc:
        with ExitStack() as ctx:
            _emit(ctx, tc, x[:], wt[:], ws[:], b[:], wc[:], out[:])
    return nc


_nc_cache = None


def _get_nc():
    global _nc_cache
    if _nc_cache is None:
        _nc_cache = _build_nc()
    return _nc_cache


def _prep_weights(weight):
    """Per-core K-major fp16 weights (exact: |w| <= 127)."""
    return [np.ascontiguousarray(weight[c * NS:(c + 1) * NS].T.astype(np.float16))
            for c in range(N_CORES)]


def kernel(x, weight, weight_scale, bias):
    x = np.ascontiguousarray(np.asarray(x, dtype=np.float32))
    weight = np.ascontiguousarray(np.asarray(weight, dtype=np.int8))
    weight_scale = np.ascontiguousarray(np.asarray(weight_scale, dtype=np.float32))
    bias = np.ascontiguousarray(np.asarray(bias, dtype=np.float32))
    assert x.shape == (M_FULL, K_DIM)
    assert weight.shape == (N_FULL, K_DIM)

    wts = _prep_weights(weight)
    nc = _get_nc()
    in_maps = []
    for c in range(N_CORES):
        sl = slice(c * NS, (c + 1) * NS)
        colsum = weight[sl].astype(np.int64).sum(axis=1).astype(np.float64)
        wc = -MAGIC16 * colsum * weight_scale[sl, 0].astype(np.float64)
        in_maps.append({
            "x": x,
            "wt": wts[c],
            "ws": weight_scale[sl],
            "b": bias[sl],
            "wc": wc.astype(np.float32),
        })
    import os
    trace = os.environ.get("BASS_TRACE") == "1"
    if trace:
        _install_ntff_hook()
    res = run_bass_kernel_spmd(nc, in_maps, core_ids=list(range(N_CORES)),
                               trace=trace)
    global LAST_EXEC_TIME_NS
    LAST_EXEC_TIME_NS = res.exec_time_ns
    out = np.concatenate([res.results[c]["out"] for c in range(N_CORES)], axis=1)
    return out.astype(np.float32)


LAST_EXEC_TIME_NS = None


def _install_ntff_hook():
    """Provide antenv.axon_hooks (missing in this image) so that
    run_bass_kernel_spmd(trace=True) can capture NTFF profiles."""
    import contextlib
    import ctypes
    import types

    try:
        from antenv.axon_hooks import get_axon_ntff_profile_hook  # noqa: F401
        return
    except ImportError:
        pass
    lib = ctypes.CDLL("/opt/axon/libaxon_pjrt.so")
    if not hasattr(lib, "axon_start_nrt_profile"):
        return
    lib.axon_start_nrt_profile.argtypes = [
        ctypes.POINTER(ctypes.c_int64), ctypes.c_size_t]
    lib.axon_start_nrt_profile.restype = ctypes.c_int64
    lib.axon_stop_nrt_profile.argtypes = [ctypes.c_char_p]
    lib.axon_stop_nrt_profile.restype = ctypes.c_int64

    @contextlib.contextmanager
    def _hook(output_dir, device_ids):
        import jax
        jax.devices()
        if device_ids:
            ids = (ctypes.c_int64 * len(device_ids))(*device_ids)
            rc = lib.axon_start_nrt_profile(ids, len(device_ids))
        else:
            rc = lib.axon_start_nrt_profile(None, 0)
        if rc != 0:
            raise RuntimeError(f"axon_start_nrt_profile rc={rc}")
        try:
            yield
        finally:
            n = lib.axon_stop_nrt_profile(str(output_dir).encode())
            import sys as _sys
            print(f"profile: {n} file(s) written to {output_dir}",
                  file=_sys.stderr)

    import antenv
    mod = types.ModuleType("antenv.axon_hooks")
    mod.get_axon_ntff_profile_hook = lambda: _hook
    mod.set_axon_ntff_profile_hook = lambda h: None
    sys.modules["antenv.axon_hooks"] = mod
    antenv.axon_hooks = mod


# revision 34
# speedup vs baseline: 1.0035x; 1.0035x over previous
"""DynamicW8A8Int8Linear on 8 Trainium2 NeuronCores (Bass/Tile).

Column-parallel: each core gets the full activation x [8192, 4096] and a
1536-wide shard of weight / weight_scale / bias; it computes its
[8192, 1536] slice of the output. No communication.

The int8 GEMM acc = x_q @ w.T runs in fp16 on the Tensor engine (exact:
x_q, w are integers, products exact in fp32 PSUM).

Per 128-token m-tile (steady state, 22.6us/tile; matmuls pace at the
216ns N=512 fp16 roofline with zero mid-kernel PE gaps):
  - DMA x tile [128, 4096] fp32 (SP HWDGE ring, which carries ONLY x in
    steady state)
  - DVE: amax = max|x| over K; xs = max(amax,1e-8)/127; inv = 1/xs
  - Act: tq = x*inv + 1536 -> fp16 in two halves (the fp16 cast rounds
    to integer+1536 exactly: spacing 1 in [1024, 2048),
    round-to-nearest-even like the reference's jnp.round)
  - PE: native transposes of tq into K-major xqT, 4 groups of 8 per PSUM
    bank (2 banks ping-pong), Act copies PSUM -> SBUF.  The groups for
    tile i+1 are WOVEN into the tail of tile i's matmul stream (after kt
    26/28/30/31) so the Act copies hide entirely under matmuls -- a
    bunched transpose burst stalls the PE ~0.8us/tile on copy latency.
    (DMA-xbar transposes measured far slower: 1.4us dispatch + 256B
    packets.)
  - PE: 96 accumulating fp16 matmuls (32 k-tiles x 3 psum banks); the
    +1536 offset rides through the GEMM
  - DVE/GpSimd epilogue (software-pipelined one tile behind, per psum
    bank): out = acc*xs*ws + wc*xs + b, where wc = -1536*colsum(w)*ws
    (host-precomputed) removes the activation offset exactly.  Out-DMAs
    dispatch from GpSimd (SWDGE) so neither HWDGE ring nor the Act queue
    ever blocks the x/tq/transpose chain.
Startup: the int8 weight shard is host-transposed to K-major and
host-cast to fp16 (12.6MB/core), loaded via HWDGE in 8 chunks ordered
x0, w0, x1, w1..w7 on the SP ring (x2/x3 go via the Act ring to dodge
the weight queue); ws/wc/bias are DMA'd as single [1,1536] rows and
broadcast to 128 partitions on-chip via K=1 fp32 matmuls (saves 2.3MB
of HBM traffic during the bandwidth-critical weight load).  25 dummy
matmuls + the 9 broadcast matmuls ramp the PE HAM clock gate (1.2 ->
2.4 GHz after ~3.4us of activity) while the first x tile streams in, so
real matmuls start warm.  The last tile runs bank-major with immediate
per-bank epilogues to shorten the kernel tail.

Buffer-lifetime note: pool bufs are sized for the software-pipelined
EMISSION order (quant(i+1) is emitted before epilogue(i-1)); xs needs 3
bufs or the scale of a tile still awaiting its epilogue is overwritten.
"""
import os

import sys
from contextlib import ExitStack

import numpy as np

for p in ("/opt/trn_rl_repo", "/opt/pypackages"):
    if p not in sys.path:
        sys.path.append(p)

import ml_dtypes
import orjson
import bass_rust
import concourse.bass as bass
import concourse.mybir as mybir
import concourse.tile as tile
from concourse.masks import make_identity
from concourse.vector_clock import ScopedClock
from concourse.bass_utils import run_bass_kernel_spmd

# ---------------------------------------------------------------------------
# Workaround for the walrus build here, which accepts at most ONE sem-wait per
# instruction ("Too many sync wait commands" in setupSyncWait): split the Tile
# end-drain at emission time, and hoist excess waits from any instruction onto
# injected same-engine NoOps at serialization time (program order on the same
# engine makes that semantically identical).
# ---------------------------------------------------------------------------
MAX_WAITS = 1


def _drain_and_barrier_split(self, tick_clock, wait_clock):
    nc = self.nc
    drain_inst = nc.sync.drain()
    wait_clock.add_sem_waits(drain_inst.ins, ScopedClock({None: tick_clock.global_clock}))
    si = drain_inst.ins.sync_info
    waits = list(si.on_wait) if si is not None and si.on_wait else []
    if len(waits) > MAX_WAITS:
        si.on_wait = waits[:MAX_WAITS]
        drain_inst.ins.sync_info = si
        rest = waits[MAX_WAITS:]
        while rest:
            extra = nc.sync.drain()
            extra.ins.sync_info = bass_rust.SyncInfo(
                on_wait=rest[:MAX_WAITS], on_update=[])
            rest = rest[MAX_WAITS:]
    nc.all_engine_barrier()
    assert self.sems is not None
    popped = nc._tile_sem_poison_stack.pop()
    assert popped is self._sem_poison
    nc.clear_and_free_semaphores(list(self.sems.allocated().values()))
    nc.all_engine_barrier()


_split_counter = [0]


def _split_waits_json(raw: bytes) -> bytes:
    j = orjson.loads(raw)
    changed = [False]

    def fix_block(b):
        ins_list = b.get("instructions")
        if ins_list:
            new_list = []
            for ins in ins_list:
                si = ins.get("sync_info")
                waits = (si or {}).get("on_wait") or []
                if len(waits) > MAX_WAITS:
                    changed[0] = True
                    for w in waits[:-MAX_WAITS]:
                        _split_counter[0] += 1
                        new_list.append({
                            "name": f"WSPLIT-{_split_counter[0]}",
                            "opcode": "NoOp",
                            "engine": ins["engine"],
                            "ins": [],
                            "outs": [],
                            "sync_info": {"on_update": [], "on_wait": [w]},
                        })
                    si["on_wait"] = waits[-MAX_WAITS:]
                new_list.append(ins)
            b["instructions"] = new_list
        for sub in (b.get("blocks") or []):
            fix_block(sub)

    for fn in j.get("functions", []):
        for b in (fn.get("blocks") or []):
            fix_block(b)
    if not changed[0]:
        return raw
    return orjson.dumps(j)


_orig_to_json_bytes = bass.Bass.to_json_bytes


def _to_json_bytes_split(self) -> bytes:
    return _split_waits_json(_orig_to_json_bytes(self))


tile.TileContext._drain_and_barrier = _drain_and_barrier_split
bass.Bass.to_json_bytes = _to_json_bytes_split

# ---------------------------------------------------------------------------
# Kernel
# ---------------------------------------------------------------------------
P = 128
MAGIC16 = 1536.0    # 1.5 * 2**10: fp16 add rounds to nearest-even integer
FREE = 512          # psum bank width (fp32)

M_FULL, K_DIM, N_FULL = 8192, 4096, 12288
N_CORES = 8
NS = N_FULL // N_CORES  # 1536 out_features per core
KT = K_DIM // P         # 32 k-tiles
NB = NS // FREE         # 3 psum banks
N_WARMUP = 25           # dummy matmuls (after the 9 broadcast matmuls) that
                        # keep the PE busy until the tile-0 quant chain lands

f32 = mybir.dt.float32
fp16 = mybir.dt.float16

ALU = mybir.AluOpType
ACT = mybir.ActivationFunctionType


def _emit(ctx: ExitStack, tc: tile.TileContext, x_ap, wt_ap, ws_ap,
          b_ap, wc_ap, out_ap):
    nc = tc.nc
    M, K = x_ap.shape
    MT = M // P

    const = ctx.enter_context(tc.tile_pool(name="const", bufs=1))

    # Per-channel scale / offset / bias: DMA only the [1, 1536] rows (18KB)
    # and broadcast to all 128 partitions ON-CHIP via K=1 fp32 matmuls with a
    # ones-vector (PE is idle at startup anyway and this doubles as HAM
    # warm-up).  A partition_broadcast DMA would pull 2.3MB through HBM right
    # when the weight load is bandwidth-critical.
    # The rows land in partition 0 of the destination tiles themselves (no
    # extra SBUF); the broadcast matmul reads row 0 before the full-tile
    # copy overwrites it.
    wsb = const.tile([P, NS], f32)
    bb = const.tile([P, NS], f32)
    wcb = const.tile([P, NS], f32)
    nc.sync.dma_start(wsb[0:1, :], ws_ap.rearrange("n o -> o n"))
    nc.sync.dma_start(bb[0:1, :], b_ap[None, :])
    nc.sync.dma_start(wcb[0:1, :], wc_ap[None, :])

    xpool = ctx.enter_context(tc.tile_pool(name="x", bufs=2))
    qpool = ctx.enter_context(tc.tile_pool(name="q", bufs=2))
    qtpool = ctx.enter_context(tc.tile_pool(name="qt", bufs=2))
    opool = ctx.enter_context(tc.tile_pool(name="o", bufs=2))
    # bufs=3: xs(mi) is read by _epilogue(mi), which is emitted one iteration
    # AFTER _quant(mi+2) allocates -- with 2 bufs the overwrite would be
    # emitted before the read and corrupt the scales.
    spool = ctx.enter_context(tc.tile_pool(name="s", bufs=3))
    mpsum = ctx.enter_context(tc.tile_pool(name="mpsum", bufs=2, space="PSUM"))
    tpsum = ctx.enter_context(tc.tile_pool(name="tpsum", bufs=2, space="PSUM"))
    ident = const.tile([P, P], fp16)
    make_identity(nc, ident)

    # Dummy data for PE warm-up matmuls.
    dummy = const.tile([P, FREE], fp16)
    nc.vector.memset(dummy[:], 1.0)

    # Prewarm the Act engine's activation table (the first ACTIVATE pays a
    # one-time ~1.3us ACT_TABLE_LOAD; do it at t~0, not on tq(0)'s critical
    # path).  Writes a scratch column so the warm-up matmuls reading `dummy`
    # aren't serialized behind it.
    prew = const.tile([P, 1], fp16)
    nc.scalar.activation(prew[:], dummy[:, 0:1], ACT.Copy)

    # SP-ring order: x0, weight chunk 0, x1, chunks 1-7.  x0 + chunk 0 gate
    # the first matmuls; the rest stream behind while tile 0 computes.
    wT = const.tile([P, KT, NS], fp16)
    wt_r = wt_ap.rearrange("(kt p) n -> p kt n", p=P)

    def _wchunk(ck):
        nc.sync.dma_start(wT[:, ck * 4:(ck + 1) * 4, :],
                          wt_r[:, ck * 4:(ck + 1) * 4, :])

    xt_pre = []
    xt0 = xpool.tile([P, K], f32, tag="xt", name="xt0")
    nc.sync.dma_start(xt0[:], x_ap[0:P, :])
    xt_pre.append(xt0)
    _wchunk(0)
    xt1 = xpool.tile([P, K], f32, tag="xt", name="xt1")
    nc.sync.dma_start(xt1[:], x_ap[P:2 * P, :])
    xt_pre.append(xt1)
    for ck in range(1, 8):
        _wchunk(ck)

    # PE warm-up: the HAM clock gate needs ~3.4us of sustained PE activity to
    # unthrottle 1.2 -> 2.4 GHz.  Run dummy matmuls while the first x tile /
    # weights are still in flight so real matmuls start warm.  The output
    # borrows an mpsum rotation slot (WAW with the first real m-tile's bank,
    # which starts later anyway) so no extra PSUM bank is needed.
    wps = mpsum.tile([P, FREE], f32, tag="mps0", name="wps")
    for wi in range(N_WARMUP):
        nc.tensor.matmul(wps[:], dummy[:, 0:P], dummy[:], start=True, stop=True)

    # On-chip broadcast of the epilogue vectors (after the dummies, so the fp32
    # matmuls run at warm clock): 9 K=1 fp32 matmuls + Act PSUM->SBUF copies.
    ones1 = const.tile([1, P], f32)
    nc.vector.memset(ones1[:], 1.0)
    for ti, dst in enumerate((wsb, bb, wcb)):
        for nb_i in range(NB):
            sl = slice(nb_i * FREE, (nb_i + 1) * FREE)
            bps = mpsum.tile([P, FREE], f32, tag=f"mps{nb_i}",
                             name=f"bc{ti}_{nb_i}")
            nc.tensor.matmul(bps[:], ones1[:], dst[0:1, sl], start=True,
                             stop=True)
            nc.scalar.activation(dst[:, sl], bps[:], ACT.Copy)

    def _epilogue_bank(pmi, pbank, pxs, nb_i):
        # out = acc * xs * ws + wc * xs + b  (wc removes the +1536 offset:
        # wc = -1536 * colsum(w) * ws, host-precomputed).  Out-DMA on the
        # gpsimd SWDGE ring: keeps the Act queue (tq + transpose copies) and
        # the SP ring (x in) free of epilogue work.
        sl = slice(nb_i * FREE, (nb_i + 1) * FREE)
        ot = opool.tile([P, FREE], f32, tag=f"ot{nb_i}", name=f"ot{pmi}_{nb_i}")
        nc.vector.scalar_tensor_tensor(
            ot[:], pbank[:], pxs[:, 0:1], wsb[:, sl],
            op0=ALU.mult, op1=ALU.mult,
        )
        nc.vector.scalar_tensor_tensor(
            ot[:], wcb[:, sl], pxs[:, 0:1], ot[:], op0=ALU.mult, op1=ALU.add,
        )
        nc.gpsimd.tensor_tensor(ot[:], ot[:], bb[:, sl], op=ALU.add)
        nc.gpsimd.dma_start(out_ap[pmi * P:(pmi + 1) * P, sl], ot[:])

    def _epilogue(pmi, pbanks, pxs):
        for nb_i in range(NB):
            _epilogue_bank(pmi, pbanks[nb_i], pxs, nb_i)

    def _quant(mi):
        """x DMA + per-token quant chain for tile mi; returns (xs, tq, xqT)."""
        if mi < 2:
            xt = xt_pre[mi]
        else:
            xt = xpool.tile([P, K], f32, tag="xt", name=f"xt{mi}")
            # mi 2/3 go via the Act HWDGE ring: on the SP ring they would
            # queue behind all 12.6MB of weight chunks and land too late for
            # the tile-1/2 transpose weave.
            eng = nc.scalar if mi in (2, 3) else nc.sync
            eng.dma_start(xt[:], x_ap[mi * P:(mi + 1) * P, :])

        # per-token quant params (DVE)
        amax = spool.tile([P, 1], f32, tag="amax", name=f"amax{mi}")
        nc.vector.tensor_reduce(
            amax[:], xt[:], axis=mybir.AxisListType.X,
            op=ALU.max, apply_absolute_value=True,
        )
        xs = spool.tile([P, 1], f32, tag="xs", name=f"xs{mi}")
        nc.vector.tensor_scalar(
            xs[:], amax[:], 1e-8, 1.0 / 127.0, op0=ALU.max, op1=ALU.mult,
        )
        inv = spool.tile([P, 1], f32, tag="inv", name=f"inv{mi}")
        nc.vector.reciprocal(inv[:], xs[:])

        # tq = x*inv + 1536 -> fp16: the fp16 cast rounds to integer+1536
        # exactly (spacing 1 in [1024, 2048)).  Two halves so the first
        # transpose groups can start before the whole row is quantized.
        tq = qpool.tile([P, K], fp16, tag="tq", name=f"tq{mi}")
        for th in range(2):
            hs = slice(th * (K // 2), (th + 1) * (K // 2))
            nc.scalar.activation(tq[:, hs], xt[:, hs], ACT.Copy, bias=MAGIC16,
                                 scale=inv[:, 0:1])
        xqT = qtpool.tile([P, KT, P], fp16, tag="xqT", name=f"xqT{mi}")
        return (xs, tq, xqT)

    def _tgroup(mi, tq, xqT, g):
        """PE-transpose k-tiles 8g..8g+7 of tq into PSUM; Act copies to xqT."""
        pt = tpsum.tile([P, 8, P], fp16, tag="tps", name=f"tps{mi}_{g}")
        for jj in range(8):
            c = g * 8 + jj
            nc.tensor.transpose(
                pt[:, jj, :], tq[:, c * P:(c + 1) * P], ident[:],
            )
        nc.scalar.activation(xqT[:, g * 8:(g + 1) * 8, :], pt[:], ACT.Copy)

    # Tile 0's quant + transposes stand alone (nothing to weave them into).
    q = {0: _quant(0)}
    for g in range(4):
        _tgroup(0, q[0][1], q[0][2], g)

    # Transpose groups of tile mi+1 are woven into the tail of tile mi's
    # matmul stream (after kt 26/28/30/31): the PE pays the 32x128-col
    # transpose cost but its Act PSUM->SBUF copies fully hide under matmuls,
    # so there is no copy-latency stall and no idle at the tile boundary.
    WEAVE_AT = {26: 0, 28: 1, 30: 2}
    prev = None
    for mi in range(MT):
        xs, tq, xqT = q.pop(mi)
        if mi + 1 < MT:
            q[mi + 1] = _quant(mi + 1)
            nxt = q[mi + 1]

        if prev is not None:
            _epilogue(*prev)

        # main GEMM (fp16 exact): acc[m, n] += xqT[p, kt, m] * wT[p, kt, n]
        banks = [
            mpsum.tile([P, FREE], f32, tag=f"mps{nb_i}", name=f"mps{mi}_{nb_i}")
            for nb_i in range(NB)
        ]
        if mi < MT - 1:
            for kt in range(KT):
                lhsT = xqT[:, kt, :]
                for nb_i in range(NB):
                    nc.tensor.matmul(
                        banks[nb_i][:], lhsT,
                        wT[:, kt, nb_i * FREE:(nb_i + 1) * FREE],
                        start=(kt == 0), stop=(kt == KT - 1),
                    )
                if kt in WEAVE_AT:
                    _tgroup(mi + 1, nxt[1], nxt[2], WEAVE_AT[kt])
            _tgroup(mi + 1, nxt[1], nxt[2], 3)
            prev = (mi, banks, xs)
        else:
            # Last tile: bank-major so each bank's epilogue + out-DMA overlaps
            # the next bank's matmuls -- shortens the kernel tail.
            for nb_i in range(NB):
                for kt in range(KT):
                    nc.tensor.matmul(
                        banks[nb_i][:], xqT[:, kt, :],
                        wT[:, kt, nb_i * FREE:(nb_i + 1) * FREE],
                        start=(kt == 0), stop=(kt == KT - 1),
                    )
                _epilogue_bank(mi, banks[nb_i], xs, nb_i)
            prev = None
    if prev is not None:
        _epilogue(*prev)


def _build_nc(m_rows=M_FULL):
    nc = bass.Bass()
    x = nc.dram_tensor("x", (m_rows, K_DIM), f32, kind="ExternalInput")
    wt = nc.dram_tensor("wt", (K_DIM, NS), fp16, kind="ExternalInput")
    ws = nc.dram_tensor("ws", (NS, 1), f32, kind="ExternalInput")
    b = nc.dram_tensor("b", (NS,), f32, kind="ExternalInput")
    wc = nc.dram_tensor("wc", (NS,), f32, kind="ExternalInput")
    out = nc.dram_tensor("out", (m_rows, NS), f32, kind="ExternalOutput")
    with tile.TileContext(nc) as tc:
        with ExitStack() as ctx:
            _emit(ctx, tc, x[:], wt[:], ws[:], b[:], wc[:], out[:])
    return nc


_nc_cache = None


def _get_nc():
    global _nc_cache
    if _nc_cache is None:
        _nc_cache = _build_nc()
    return _nc_cache


def _prep_weights(weight):
    """Per-core K-major fp16 weights (exact: |w| <= 127)."""
    return [np.ascontiguousarray(weight[c * NS:(c + 1) * NS].T.astype(np.float16))
            for c in range(N_CORES)]


def kernel(x, weight, weight_scale, bias):
    x = np.ascontiguousarray(np.asarray(x, dtype=np.float32))
    weight = np.ascontiguousarray(np.asarray(weight, dtype=np.int8))
    weight_scale = np.ascontiguousarray(np.asarray(weight_scale, dtype=np.float32))
    bias = np.ascontiguousarray(np.asarray(bias, dtype=np.float32))
    assert x.shape == (M_FULL, K_DIM)
    assert weight.shape == (N_FULL, K_DIM)

    wts = _prep_weights(weight)
    nc = _get_nc()
    in_maps = []
    for c in range(N_CORES):
        sl = slice(c * NS, (c + 1) * NS)
        colsum = weight[sl].astype(np.int64).sum(axis=1).astype(np.float64)
        wc = -MAGIC16 * colsum * weight_scale[sl, 0].astype(np.float64)
        in_maps.append({
            "x": x,
            "wt": wts[c],
            "ws": weight_scale[sl],
            "b": bias[sl],
            "wc": wc.astype(np.float32),
        })
    import os
    trace = os.environ.get("BASS_TRACE") == "1"
    if trace:
        _install_ntff_hook()
    res = run_bass_kernel_spmd(nc, in_maps, core_ids=list(range(N_CORES)),
                               trace=trace)
    global LAST_EXEC_TIME_NS
    LAST_EXEC_TIME_NS = res.exec_time_ns
    out = np.concatenate([res.results[c]["out"] for c in range(N_CORES)], axis=1)
    return out.astype(np.float32)


LAST_EXEC_TIME_NS = None


def _install_ntff_hook():
    """Provide antenv.axon_hooks (missing in this image) so that
    run_bass_kernel_spmd(trace=True) can capture NTFF profiles."""
    import contextlib
    import ctypes
    import types

    try:
        from antenv.axon_hooks import get_axon_ntff_profile_hook  # noqa: F401
        return
    except ImportError:
        pass
    lib = ctypes.CDLL("/opt/axon/libaxon_pjrt.so")
    if not hasattr(lib, "axon_start_nrt_profile"):
        return
    lib.axon_start_nrt_profile.argtypes = [
        ctypes.POINTER(ctypes.c_int64), ctypes.c_size_t]
    lib.axon_start_nrt_profile.restype = ctypes.c_int64
    lib.axon_stop_nrt_profile.argtypes = [ctypes.c_char_p]
    lib.axon_stop_nrt_profile.restype = ctypes.c_int64

    @contextlib.contextmanager
    def _hook(output_dir, device_ids):
        import jax
        jax.devices()
        if device_ids:
            ids = (ctypes.c_int64 * len(device_ids))(*device_ids)
            rc = lib.axon_start_nrt_profile(ids, len(device_ids))
        else:
            rc = lib.axon_start_nrt_profile(None, 0)
        if rc != 0:
            raise RuntimeError(f"axon_start_nrt_profile rc={rc}")
        try:
            yield
        finally:
            n = lib.axon_stop_nrt_profile(str(output_dir).encode())
            import sys as _sys
            print(f"profile: {n} file(s) written to {output_dir}",
                  file=_sys.stderr)

    import antenv
    mod = types.ModuleType("antenv.axon_hooks")
    mod.get_axon_ntff_profile_hook = lambda: _hook
    mod.set_axon_ntff_profile_hook = lambda h: None
    sys.modules["antenv.axon_hooks"] = mod
    antenv.axon_hooks = mod


# revision 35
# speedup vs baseline: 1.0063x; 1.0027x over previous
"""DynamicW8A8Int8Linear on 8 Trainium2 NeuronCores (Bass/Tile).

Column-parallel: each core gets the full activation x [8192, 4096] and a
1536-wide shard of weight / weight_scale / bias; it computes its
[8192, 1536] slice of the output. No communication.

The int8 GEMM acc = x_q @ w.T runs in fp16 on the Tensor engine (exact:
x_q, w are integers, products exact in fp32 PSUM).

Per 128-token m-tile (steady state, 22.6us/tile; matmuls pace at the
216ns N=512 fp16 roofline with zero mid-kernel PE gaps):
  - DMA x tile [128, 4096] fp32 (SP HWDGE ring, which carries ONLY x in
    steady state)
  - DVE: amax = max|x| over K; xs = max(amax,1e-8)/127; inv = 1/xs
  - Act: tq = x*inv + 1536 -> fp16 in two halves (the fp16 cast rounds
    to integer+1536 exactly: spacing 1 in [1024, 2048),
    round-to-nearest-even like the reference's jnp.round)
  - PE: native transposes of tq into K-major xqT, 4 groups of 8 per PSUM
    bank (2 banks ping-pong), Act copies PSUM -> SBUF.  The groups for
    tile i+1 are WOVEN into the tail of tile i's matmul stream (after kt
    26/28/30/31) so the Act copies hide entirely under matmuls -- a
    bunched transpose burst stalls the PE ~0.8us/tile on copy latency.
    (DMA-xbar transposes measured far slower: 1.4us dispatch + 256B
    packets.)
  - PE: 96 accumulating fp16 matmuls (32 k-tiles x 3 psum banks); the
    +1536 offset rides through the GEMM
  - DVE/GpSimd epilogue (software-pipelined one tile behind, per psum
    bank): out = acc*xs*ws + wc*xs + b, where wc = -1536*colsum(w)*ws
    (host-precomputed) removes the activation offset exactly.  Out-DMAs
    dispatch from GpSimd (SWDGE) so neither HWDGE ring nor the Act queue
    ever blocks the x/tq/transpose chain.
Startup: the int8 weight shard is host-transposed to K-major and
host-cast to fp16 (12.6MB/core), loaded via HWDGE in 8 chunks ordered
x0, w0, x1, w1..w7 on the SP ring (x2/x3 go via the Act ring to dodge
the weight queue); ws/wc/bias are DMA'd as single [1,1536] rows and
broadcast to 128 partitions on-chip via K=1 fp32 matmuls (saves 2.3MB
of HBM traffic during the bandwidth-critical weight load).  25 dummy
matmuls + the 9 broadcast matmuls ramp the PE HAM clock gate (1.2 ->
2.4 GHz after ~3.4us of activity) while the first x tile streams in, so
real matmuls start warm.  The last tile runs bank-major with immediate
per-bank epilogues to shorten the kernel tail.

Buffer-lifetime note: pool bufs are sized for the software-pipelined
EMISSION order (quant(i+1) is emitted before epilogue(i-1)); xs needs 3
bufs or the scale of a tile still awaiting its epilogue is overwritten.
"""
import os

import sys
from contextlib import ExitStack

import numpy as np

for p in ("/opt/trn_rl_repo", "/opt/pypackages"):
    if p not in sys.path:
        sys.path.append(p)

import ml_dtypes
import orjson
import bass_rust
import concourse.bass as bass
import concourse.mybir as mybir
import concourse.tile as tile
from concourse.masks import make_identity
from concourse.vector_clock import ScopedClock
from concourse.bass_utils import run_bass_kernel_spmd

# ---------------------------------------------------------------------------
# Workaround for the walrus build here, which accepts at most ONE sem-wait per
# instruction ("Too many sync wait commands" in setupSyncWait): split the Tile
# end-drain at emission time, and hoist excess waits from any instruction onto
# injected same-engine NoOps at serialization time (program order on the same
# engine makes that semantically identical).
# ---------------------------------------------------------------------------
MAX_WAITS = 1


def _drain_and_barrier_split(self, tick_clock, wait_clock):
    nc = self.nc
    drain_inst = nc.sync.drain()
    wait_clock.add_sem_waits(drain_inst.ins, ScopedClock({None: tick_clock.global_clock}))
    si = drain_inst.ins.sync_info
    waits = list(si.on_wait) if si is not None and si.on_wait else []
    if len(waits) > MAX_WAITS:
        si.on_wait = waits[:MAX_WAITS]
        drain_inst.ins.sync_info = si
        rest = waits[MAX_WAITS:]
        while rest:
            extra = nc.sync.drain()
            extra.ins.sync_info = bass_rust.SyncInfo(
                on_wait=rest[:MAX_WAITS], on_update=[])
            rest = rest[MAX_WAITS:]
    nc.all_engine_barrier()
    assert self.sems is not None
    popped = nc._tile_sem_poison_stack.pop()
    assert popped is self._sem_poison
    nc.clear_and_free_semaphores(list(self.sems.allocated().values()))
    nc.all_engine_barrier()


_split_counter = [0]


def _split_waits_json(raw: bytes) -> bytes:
    j = orjson.loads(raw)
    changed = [False]

    def fix_block(b):
        ins_list = b.get("instructions")
        if ins_list:
            new_list = []
            for ins in ins_list:
                si = ins.get("sync_info")
                waits = (si or {}).get("on_wait") or []
                if len(waits) > MAX_WAITS:
                    changed[0] = True
                    for w in waits[:-MAX_WAITS]:
                        _split_counter[0] += 1
                        new_list.append({
                            "name": f"WSPLIT-{_split_counter[0]}",
                            "opcode": "NoOp",
                            "engine": ins["engine"],
                            "ins": [],
                            "outs": [],
                            "sync_info": {"on_update": [], "on_wait": [w]},
                        })
                    si["on_wait"] = waits[-MAX_WAITS:]
                new_list.append(ins)
            b["instructions"] = new_list
        for sub in (b.get("blocks") or []):
            fix_block(sub)

    for fn in j.get("functions", []):
        for b in (fn.get("blocks") or []):
            fix_block(b)
    if not changed[0]:
        return raw
    return orjson.dumps(j)


_orig_to_json_bytes = bass.Bass.to_json_bytes


def _to_json_bytes_split(self) -> bytes:
    return _split_waits_json(_orig_to_json_bytes(self))


tile.TileContext._drain_and_barrier = _drain_and_barrier_split
bass.Bass.to_json_bytes = _to_json_bytes_split

# ---------------------------------------------------------------------------
# Kernel
# ---------------------------------------------------------------------------
P = 128
MAGIC16 = 1536.0    # 1.5 * 2**10: fp16 add rounds to nearest-even integer
FREE = 512          # psum bank width (fp32)

M_FULL, K_DIM, N_FULL = 8192, 4096, 12288
N_CORES = 8
NS = N_FULL // N_CORES  # 1536 out_features per core
KT = K_DIM // P         # 32 k-tiles
NB = NS // FREE         # 3 psum banks
N_WARMUP = 25           # dummy matmuls (after the 9 broadcast matmuls) that
                        # keep the PE busy until the tile-0 quant chain lands

f32 = mybir.dt.float32
fp16 = mybir.dt.float16

ALU = mybir.AluOpType
ACT = mybir.ActivationFunctionType


def _emit(ctx: ExitStack, tc: tile.TileContext, x_ap, wt_ap, ws_ap,
          b_ap, wc_ap, out_ap):
    nc = tc.nc
    M, K = x_ap.shape
    MT = M // P

    const = ctx.enter_context(tc.tile_pool(name="const", bufs=1))

    # Per-channel scale / offset / bias: DMA only the [1, 1536] rows (18KB)
    # and broadcast to all 128 partitions ON-CHIP via K=1 fp32 matmuls with a
    # ones-vector (PE is idle at startup anyway and this doubles as HAM
    # warm-up).  A partition_broadcast DMA would pull 2.3MB through HBM right
    # when the weight load is bandwidth-critical.
    # The rows land in partition 0 of the destination tiles themselves (no
    # extra SBUF); the broadcast matmul reads row 0 before the full-tile
    # copy overwrites it.
    wsb = const.tile([P, NS], f32)
    bb = const.tile([P, NS], f32)
    wcb = const.tile([P, NS], f32)
    nc.sync.dma_start(wsb[0:1, :], ws_ap.rearrange("n o -> o n"))
    nc.sync.dma_start(bb[0:1, :], b_ap[None, :])
    nc.sync.dma_start(wcb[0:1, :], wc_ap[None, :])

    xpool = ctx.enter_context(tc.tile_pool(name="x", bufs=2))
    qpool = ctx.enter_context(tc.tile_pool(name="q", bufs=2))
    qtpool = ctx.enter_context(tc.tile_pool(name="qt", bufs=2))
    opool = ctx.enter_context(tc.tile_pool(name="o", bufs=2))
    # bufs=3: xs(mi) is read by _epilogue(mi), which is emitted one iteration
    # AFTER _quant(mi+2) allocates -- with 2 bufs the overwrite would be
    # emitted before the read and corrupt the scales.
    spool = ctx.enter_context(tc.tile_pool(name="s", bufs=3))
    mpsum = ctx.enter_context(tc.tile_pool(name="mpsum", bufs=2, space="PSUM"))
    tpsum = ctx.enter_context(tc.tile_pool(name="tpsum", bufs=2, space="PSUM"))
    ident = const.tile([P, P], fp16)
    make_identity(nc, ident)

    # Dummy data for PE warm-up matmuls.
    dummy = const.tile([P, FREE], fp16)
    nc.vector.memset(dummy[:], 1.0)

    # Prewarm the Act engine's activation table (the first ACTIVATE pays a
    # one-time ~1.3us ACT_TABLE_LOAD; do it at t~0, not on tq(0)'s critical
    # path).  Writes a scratch column so the warm-up matmuls reading `dummy`
    # aren't serialized behind it.
    prew = const.tile([P, 1], fp16)
    nc.scalar.activation(prew[:], dummy[:, 0:1], ACT.Copy)

    # SP-ring order: x0, weight chunk 0, x1, chunks 1-7.  x0 + chunk 0 gate
    # the first matmuls; the rest stream behind while tile 0 computes.
    wT = const.tile([P, KT, NS], fp16)
    wt_r = wt_ap.rearrange("(kt p) n -> p kt n", p=P)

    def _wchunk(ck):
        nc.sync.dma_start(wT[:, ck * 4:(ck + 1) * 4, :],
                          wt_r[:, ck * 4:(ck + 1) * 4, :])

    xt_pre = []
    xt0 = xpool.tile([P, K], f32, tag="xt", name="xt0")
    nc.sync.dma_start(xt0[:], x_ap[0:P, :])
    xt_pre.append(xt0)
    _wchunk(0)
    xt1 = xpool.tile([P, K], f32, tag="xt", name="xt1")
    nc.sync.dma_start(xt1[:], x_ap[P:2 * P, :])
    xt_pre.append(xt1)
    for ck in range(1, 8):
        _wchunk(ck)

    # PE warm-up: the HAM clock gate needs ~3.4us of sustained PE activity to
    # unthrottle 1.2 -> 2.4 GHz.  Run dummy matmuls while the first x tile /
    # weights are still in flight so real matmuls start warm.  The output
    # borrows an mpsum rotation slot (WAW with the first real m-tile's bank,
    # which starts later anyway) so no extra PSUM bank is needed.
    wps = mpsum.tile([P, FREE], f32, tag="mps0", name="wps")
    for wi in range(N_WARMUP):
        nc.tensor.matmul(wps[:], dummy[:, 0:P], dummy[:], start=True, stop=True)

    # On-chip broadcast of the epilogue vectors (after the dummies, so the fp32
    # matmuls run at warm clock): 9 K=1 fp32 matmuls + Act PSUM->SBUF copies.
    ones1 = const.tile([1, P], f32)
    nc.vector.memset(ones1[:], 1.0)
    for ti, dst in enumerate((wsb, bb, wcb)):
        for nb_i in range(NB):
            sl = slice(nb_i * FREE, (nb_i + 1) * FREE)
            bps = mpsum.tile([P, FREE], f32, tag=f"mps{nb_i}",
                             name=f"bc{ti}_{nb_i}")
            nc.tensor.matmul(bps[:], ones1[:], dst[0:1, sl], start=True,
                             stop=True)
            nc.scalar.activation(dst[:, sl], bps[:], ACT.Copy)

    def _epilogue_bank(pmi, pbank, pxs, nb_i):
        # out = acc * xs * ws + wc * xs + b  (wc removes the +1536 offset:
        # wc = -1536 * colsum(w) * ws, host-precomputed).  Out-DMA on the
        # gpsimd SWDGE ring: keeps the Act queue (tq + transpose copies) and
        # the SP ring (x in) free of epilogue work.
        sl = slice(nb_i * FREE, (nb_i + 1) * FREE)
        ot = opool.tile([P, FREE], f32, tag=f"ot{nb_i}", name=f"ot{pmi}_{nb_i}")
        nc.vector.scalar_tensor_tensor(
            ot[:], pbank[:], pxs[:, 0:1], wsb[:, sl],
            op0=ALU.mult, op1=ALU.mult,
        )
        nc.vector.scalar_tensor_tensor(
            ot[:], wcb[:, sl], pxs[:, 0:1], ot[:], op0=ALU.mult, op1=ALU.add,
        )
        nc.gpsimd.tensor_tensor(ot[:], ot[:], bb[:, sl], op=ALU.add)
        nc.gpsimd.dma_start(out_ap[pmi * P:(pmi + 1) * P, sl], ot[:])

    def _epilogue(pmi, pbanks, pxs):
        for nb_i in range(NB):
            _epilogue_bank(pmi, pbanks[nb_i], pxs, nb_i)

    def _quant(mi):
        """x DMA + per-token quant chain for tile mi; returns (xs, tq, xqT)."""
        if mi < 2:
            xt = xt_pre[mi]
        else:
            xt = xpool.tile([P, K], f32, tag="xt", name=f"xt{mi}")
            # mi 2/3 go via the Act HWDGE ring: on the SP ring they would
            # queue behind all 12.6MB of weight chunks and land too late for
            # the tile-1/2 transpose weave.
            eng = nc.scalar if mi in (2, 3) else nc.sync
            eng.dma_start(xt[:], x_ap[mi * P:(mi + 1) * P, :])

        # per-token quant params (DVE)
        amax = spool.tile([P, 1], f32, tag="amax", name=f"amax{mi}")
        nc.vector.tensor_reduce(
            amax[:], xt[:], axis=mybir.AxisListType.X,
            op=ALU.max, apply_absolute_value=True,
        )
        xs = spool.tile([P, 1], f32, tag="xs", name=f"xs{mi}")
        nc.vector.tensor_scalar(
            xs[:], amax[:], 1e-8, 1.0 / 127.0, op0=ALU.max, op1=ALU.mult,
        )
        inv = spool.tile([P, 1], f32, tag="inv", name=f"inv{mi}")
        nc.vector.reciprocal(inv[:], xs[:])

        # tq = x*inv + 1536 -> fp16: the fp16 cast rounds to integer+1536
        # exactly (spacing 1 in [1024, 2048)).  Two halves so the first
        # transpose groups can start before the whole row is quantized.
        tq = qpool.tile([P, K], fp16, tag="tq", name=f"tq{mi}")
        for th in range(2):
            hs = slice(th * (K // 2), (th + 1) * (K // 2))
            nc.scalar.activation(tq[:, hs], xt[:, hs], ACT.Copy, bias=MAGIC16,
                                 scale=inv[:, 0:1])
        xqT = qtpool.tile([P, KT, P], fp16, tag="xqT", name=f"xqT{mi}")
        return (xs, tq, xqT)

    def _tgroup(mi, tq, xqT, g):
        """PE-transpose k-tiles 8g..8g+7 of tq into PSUM; Act copies to xqT."""
        pt = tpsum.tile([P, 8, P], fp16, tag="tps", name=f"tps{mi}_{g}")
        for jj in range(8):
            c = g * 8 + jj
            nc.tensor.transpose(
                pt[:, jj, :], tq[:, c * P:(c + 1) * P], ident[:],
            )
        nc.scalar.activation(xqT[:, g * 8:(g + 1) * 8, :], pt[:], ACT.Copy)

    def _mmkt(banks, xqT, kt):
        for nb_i in range(NB):
            nc.tensor.matmul(
                banks[nb_i][:], xqT[:, kt, :],
                wT[:, kt, nb_i * FREE:(nb_i + 1) * FREE],
                start=(kt == 0), stop=(kt == KT - 1),
            )

    # ---- Tiles 0 and 1: the weight-load-bound phase, interleaved ----
    # Weight chunks stream in at ~4.4us each but one tile only consumes
    # 2.6us of matmuls per chunk, so a single tile stalls ~2us per chunk
    # (and HAM re-throttles).  Instead: run tile-0 kt0-15 as chunks 0-3
    # land, then tile-1 kt0-15 (weights already resident) while chunks 4-7
    # stream, then pair both tiles' kt16-31 at PE rate -- the PE never
    # idles and the whole startup is bandwidth-, not latency-, shaped.
    q = {0: _quant(0)}
    for g in range(4):
        _tgroup(0, q[0][1], q[0][2], g)
    q[1] = _quant(1)
    xs0, tq0, xqT0 = q.pop(0)
    xs1, tq1, xqT1 = q[1]
    banks0 = [mpsum.tile([P, FREE], f32, tag=f"mps{nb_i}", name=f"mps0_{nb_i}")
              for nb_i in range(NB)]
    banks1 = [mpsum.tile([P, FREE], f32, tag=f"mps{nb_i}", name=f"mps1_{nb_i}")
              for nb_i in range(NB)]
    for kt in range(16):
        _mmkt(banks0, xqT0, kt)
        if kt in (11, 13, 15):
            _tgroup(1, tq1, xqT1, {11: 0, 13: 1, 15: 2}[kt])
    _tgroup(1, tq1, xqT1, 3)
    q[2] = _quant(2)
    for kt in range(16):
        _mmkt(banks1, xqT1, kt)
    for kt in range(16, KT):
        _mmkt(banks0, xqT0, kt)
        _mmkt(banks1, xqT1, kt)
        if kt in (26, 28, 30):
            _tgroup(2, q[2][1], q[2][2], {26: 0, 28: 1, 30: 2}[kt])
    _tgroup(2, q[2][1], q[2][2], 3)
    _epilogue(0, banks0, xs0)
    q.pop(1)

    # Transpose groups of tile mi+1 are woven into the tail of tile mi's
    # matmul stream (after kt 26/28/30/31): the PE pays the 32x128-col
    # transpose cost but its Act PSUM->SBUF copies fully hide under matmuls,
    # so there is no copy-latency stall and no idle at the tile boundary.
    WEAVE_AT = {26: 0, 28: 1, 30: 2}
    prev = (1, banks1, xs1)
    for mi in range(2, MT):
        xs, tq, xqT = q.pop(mi)
        if mi + 1 < MT:
            q[mi + 1] = _quant(mi + 1)
            nxt = q[mi + 1]

        if prev is not None:
            _epilogue(*prev)

        # main GEMM (fp16 exact): acc[m, n] += xqT[p, kt, m] * wT[p, kt, n]
        banks = [
            mpsum.tile([P, FREE], f32, tag=f"mps{nb_i}", name=f"mps{mi}_{nb_i}")
            for nb_i in range(NB)
        ]
        if mi < MT - 1:
            for kt in range(KT):
                lhsT = xqT[:, kt, :]
                for nb_i in range(NB):
                    nc.tensor.matmul(
                        banks[nb_i][:], lhsT,
                        wT[:, kt, nb_i * FREE:(nb_i + 1) * FREE],
                        start=(kt == 0), stop=(kt == KT - 1),
                    )
                if kt in WEAVE_AT:
                    _tgroup(mi + 1, nxt[1], nxt[2], WEAVE_AT[kt])
            _tgroup(mi + 1, nxt[1], nxt[2], 3)
            prev = (mi, banks, xs)
        else:
            # Last tile: bank-major so each bank's epilogue + out-DMA overlaps
            # the next bank's matmuls -- shortens the kernel tail.
            for nb_i in range(NB):
                for kt in range(KT):
                    nc.tensor.matmul(
                        banks[nb_i][:], xqT[:, kt, :],
                        wT[:, kt, nb_i * FREE:(nb_i + 1) * FREE],
                        start=(kt == 0), stop=(kt == KT - 1),
                    )
                _epilogue_bank(mi, banks[nb_i], xs, nb_i)
            prev = None
    if prev is not None:
        _epilogue(*prev)


def _build_nc(m_rows=M_FULL):
    nc = bass.Bass()
    x = nc.dram_tensor("x", (m_rows, K_DIM), f32, kind="ExternalInput")
    wt = nc.dram_tensor("wt", (K_DIM, NS), fp16, kind="ExternalInput")
    ws = nc.dram_tensor("ws", (NS, 1), f32, kind="ExternalInput")
    b = nc.dram_tensor("b", (NS,), f32, kind="ExternalInput")
    wc = nc.dram_tensor("wc", (NS,), f32, kind="ExternalInput")
    out = nc.dram_tensor("out", (m_rows, NS), f32, kind="ExternalOutput")
    with tile.TileContext(nc) as tc:
        with ExitStack() as ctx:
            _emit(ctx, tc, x[:], wt[:], ws[:], b[:], wc[:], out[:])
    return nc


_nc_cache = None


def _get_nc():
    global _nc_cache
    if _nc_cache is None:
        _nc_cache = _build_nc()
    return _nc_cache


def _prep_weights(weight):
    """Per-core K-major fp16 weights (exact: |w| <= 127)."""
    return [np.ascontiguousarray(weight[c * NS:(c + 1) * NS].T.astype(np.float16))
            for c in range(N_CORES)]


def kernel(x, weight, weight_scale, bias):
    x = np.ascontiguousarray(np.asarray(x, dtype=np.float32))
    weight = np.ascontiguousarray(np.asarray(weight, dtype=np.int8))
    weight_scale = np.ascontiguousarray(np.asarray(weight_scale, dtype=np.float32))
    bias = np.ascontiguousarray(np.asarray(bias, dtype=np.float32))
    assert x.shape == (M_FULL, K_DIM)
    assert weight.shape == (N_FULL, K_DIM)

    wts = _prep_weights(weight)
    nc = _get_nc()
    in_maps = []
    for c in range(N_CORES):
        sl = slice(c * NS, (c + 1) * NS)
        colsum = weight[sl].astype(np.int64).sum(axis=1).astype(np.float64)
        wc = -MAGIC16 * colsum * weight_scale[sl, 0].astype(np.float64)
        in_maps.append({
            "x": x,
            "wt": wts[c],
            "ws": weight_scale[sl],
            "b": bias[sl],
            "wc": wc.astype(np.float32),
        })
    import os
    trace = os.environ.get("BASS_TRACE") == "1"
    if trace:
        _install_ntff_hook()
    res = run_bass_kernel_spmd(nc, in_maps, core_ids=list(range(N_CORES)),
                               trace=trace)
    global LAST_EXEC_TIME_NS
    LAST_EXEC_TIME_NS = res.exec_time_ns
    out = np.concatenate([res.results[c]["out"] for c in range(N_CORES)], axis=1)
    return out.astype(np.float32)


LAST_EXEC_TIME_NS = None


def _install_ntff_hook():
    """Provide antenv.axon_hooks (missing in this image) so that
    run_bass_kernel_spmd(trace=True) can capture NTFF profiles."""
    import contextlib
    import ctypes
    import types

    try:
        from antenv.axon_hooks import get_axon_ntff_profile_hook  # noqa: F401
        return
    except ImportError:
        pass
    lib = ctypes.CDLL("/opt/axon/libaxon_pjrt.so")
    if not hasattr(lib, "axon_start_nrt_profile"):
        return
    lib.axon_start_nrt_profile.argtypes = [
        ctypes.POINTER(ctypes.c_int64), ctypes.c_size_t]
    lib.axon_start_nrt_profile.restype = ctypes.c_int64
    lib.axon_stop_nrt_profile.argtypes = [ctypes.c_char_p]
    lib.axon_stop_nrt_profile.restype = ctypes.c_int64

    @contextlib.contextmanager
    def _hook(output_dir, device_ids):
        import jax
        jax.devices()
        if device_ids:
            ids = (ctypes.c_int64 * len(device_ids))(*device_ids)
            rc = lib.axon_start_nrt_profile(ids, len(device_ids))
        else:
            rc = lib.axon_start_nrt_profile(None, 0)
        if rc != 0:
            raise RuntimeError(f"axon_start_nrt_profile rc={rc}")
        try:
            yield
        finally:
            n = lib.axon_stop_nrt_profile(str(output_dir).encode())
            import sys as _sys
            print(f"profile: {n} file(s) written to {output_dir}",
                  file=_sys.stderr)

    import antenv
    mod = types.ModuleType("antenv.axon_hooks")
    mod.get_axon_ntff_profile_hook = lambda: _hook
    mod.set_axon_ntff_profile_hook = lambda h: None
    sys.modules["antenv.axon_hooks"] = mod
    antenv.axon_hooks = mod
